# revision 9
# baseline (speedup 1.0000x reference)
"""Multi-head attention (B=2,S=2048,D=1024,H=16) on 8 trn2 NeuronCores.

Sharding: core = b*4 + g  (b = batch 0..1, g = head-group 0..3, 4 heads each).
Each core computes QKV projections for its 256 output dims, causal attention
for its 4 heads (scores kept transposed: [s_k, s_q]), and a K-sliced partial
of the output projection (transposed: [D, S]).  Host sums the 4 partials per
batch and adds b_o.

All matmuls in bf16 (fp32 PSUM accumulate); softmax without max-subtraction
(scores/8 are small, exp cannot overflow); sumexp via an all-ones [128,64]
stationary matmul that also broadcasts the sum to all partitions of each
head's half, so normalization is a plain elementwise multiply.
"""
import sys

if "/opt/trn_rl_repo" not in sys.path:
    sys.path.insert(0, "/opt/trn_rl_repo")

import numpy as np
import ml_dtypes

B, S, D, H = 2, 2048, 1024, 16
HD = D // H            # 64
G = 4                  # head groups (one per core within a batch)
HPG = H // G           # 4 heads per group
DG = HPG * HD          # 256 dims per group
SCALE = 8.0
NCORES = 8
NQC = S // 512         # 4 query chunks
NJ = S // 128          # 16 key tiles
KC = D // 128          # 8 contraction chunks
BF16 = ml_dtypes.bfloat16

_CACHE = {}


def _build(causal: bool):
    import concourse.mybir as mybir
    import concourse.tile as tile
    from concourse import bacc

    f32 = mybir.dt.float32
    b16 = mybir.dt.bfloat16
    Exp = mybir.ActivationFunctionType.Exp

    nc = bacc.Bacc(None, target_bir_lowering=False)

    qT = nc.dram_tensor("qT", [D, S], b16, kind="ExternalInput")
    kT = nc.dram_tensor("kT", [D, S], b16, kind="ExternalInput")
    vT = nc.dram_tensor("vT", [D, S], b16, kind="ExternalInput")
    wqT = nc.dram_tensor("wqT", [D, DG], b16, kind="ExternalInput")
    wkT = nc.dram_tensor("wkT", [D, DG], b16, kind="ExternalInput")
    wvT = nc.dram_tensor("wvT", [D, DG], b16, kind="ExternalInput")
    woT = nc.dram_tensor("woT", [DG, D], b16, kind="ExternalInput")
    bq = nc.dram_tensor("bq", [DG, 1], f32, kind="ExternalInput")
    bk = nc.dram_tensor("bk", [DG, 1], f32, kind="ExternalInput")
    bv = nc.dram_tensor("bv", [1, DG], b16, kind="ExternalInput")
    tri = nc.dram_tensor("tri", [128, 128], b16, kind="ExternalInput")
    out = nc.dram_tensor("out", [D, S], f32, kind="ExternalOutput")

    with tile.TileContext(nc) as tc:
        with (
            tc.tile_pool(name="consts", bufs=1) as consts,
            tc.tile_pool(name="proj", bufs=1) as proj,
            tc.tile_pool(name="pin", bufs=1) as pin,
            tc.tile_pool(name="probs", bufs=6) as probsp,
            tc.tile_pool(name="rec", bufs=2) as recp,
            tc.tile_pool(name="ost", bufs=3) as ostp,
            tc.tile_pool(name="mm", bufs=3, space="PSUM") as mmp,
            tc.tile_pool(name="cpsum", bufs=1, space="PSUM") as cpsum,
            tc.tile_pool(name="upsum", bufs=1, space="PSUM") as upsum,
        ):
            # --- constant tiles -------------------------------------------
            wq_t = consts.tile([128, KC * DG], b16)
            wk_t = consts.tile([128, KC * DG], b16)
            wv_t = consts.tile([128, KC * DG], b16)
            wo_t = consts.tile([128, 2 * D], b16)
            bq_t = consts.tile([128, 2], f32)
            bk_t = consts.tile([128, 2], f32)
            bv_t = consts.tile([1, DG], b16)
            tri_t = consts.tile([128, 128], b16)
            ones64_t = consts.tile([128, HD], b16)
            nc.vector.memset(ones64_t[:], 1.0)
            ones1_t = consts.tile([1, 128], b16)
            nc.vector.memset(ones1_t[:], 1.0)
            warm_sb = consts.tile([128, 128], b16)
            nc.vector.memset(warm_sb[:], 0.0)

            # --- persistent projection outputs ----------------------------
            # qpT/kpT: pair p in cols [p*S,(p+1)*S); rows 0:64 head 2p, 64:128 head 2p+1
            qpT = proj.tile([128, 2 * S], b16)
            kpT = proj.tile([128, 2 * S], b16)
            # vp: key tile j in cols [j*DG,(j+1)*DG); within: local head hh at 64*hh
            vp = proj.tile([128, NJ * DG], b16)
            # ctxT: same pair layout as qpT, normalized attention output (c x s)
            ctxT = proj.tile([128, 2 * S], b16)

            # --- input row tiles + DMA schedule ---------------------------
            # quarter-split column DMAs so compute starts as soon as the
            # first 512 columns of each row land
            qrow = [pin.tile([128, S], b16, name=f"qrow{kc}") for kc in range(KC)]
            krow = [pin.tile([128, S], b16, name=f"krow{kc}") for kc in range(KC)]
            vrow = [pin.tile([128, S], b16, name=f"vrow{kc}") for kc in range(KC)]

            def dma_w(w_t, src_w):
                for kc in range(KC):
                    nc.sync.dma_start(w_t[:, kc * DG:(kc + 1) * DG], src_w[kc * 128:(kc + 1) * 128, :])

            def dma_rows(rows, src, n):
                cs = slice(n * 512, (n + 1) * 512)
                for kc in range(KC):
                    nc.sync.dma_start(rows[kc][:, cs], src[kc * 128:(kc + 1) * 128, cs])

            dma_w(wq_t, wqT)
            dma_rows(qrow, qT, 0)
            dma_w(wk_t, wkT)
            dma_rows(krow, kT, 0)
            dma_w(wv_t, wvT)
            dma_rows(vrow, vT, 0)
            for m2 in range(2):
                nc.sync.dma_start(bq_t[:, m2:m2 + 1], bq[m2 * 128:(m2 + 1) * 128, :])
                nc.sync.dma_start(bk_t[:, m2:m2 + 1], bk[m2 * 128:(m2 + 1) * 128, :])
            nc.sync.dma_start(bv_t[:], bv[:])
            nc.sync.dma_start(tri_t[:], tri[:])
            for p2 in range(2):
                nc.sync.dma_start(wo_t[:, p2 * D:(p2 + 1) * D], woT[p2 * 128:(p2 + 1) * 128, :])
            for n in range(1, NQC):
                dma_rows(qrow, qT, n)
                dma_rows(krow, kT, n)
                dma_rows(vrow, vT, n)

            # warmup burst: keeps the PE activity monitor at full clock
            # while the first input quarters stream in
            warm_ps = mmp.tile([128, 1024], f32, tag="sc", name="warm")
            for wi in range(36):
                nc.tensor.matmul(warm_ps[:, 0:128], warm_sb[:], warm_sb[:],
                                 start=(wi == 0), stop=(wi == 35))

            # --- projections, by column quarter ---------------------------
            for n in range(NQC):
                for name, rows, w_t, dst, bias_t in (
                    ("q", qrow, wq_t, qpT, bq_t),
                    ("k", krow, wk_t, kpT, bk_t),
                ):
                    for m in range(2):
                        ps = mmp.tile([128, 1024], f32, tag="sc", name=f"{name}ps{m}{n}")
                        for kc in range(KC):
                            nc.tensor.matmul(
                                ps[:, 0:512],
                                w_t[:, kc * DG + m * 128: kc * DG + (m + 1) * 128],
                                rows[kc][:, n * 512:(n + 1) * 512],
                                start=(kc == 0), stop=(kc == KC - 1),
                            )
                        nc.vector.tensor_scalar_add(
                            dst[:, m * S + n * 512: m * S + (n + 1) * 512],
                            ps[:, 0:512], bias_t[:, m:m + 1],
                        )
                for j in range(4 * n, 4 * n + 4):
                    ps = mmp.tile([128, 1024], f32, tag="sc", name=f"vps{j}")
                    for kc in range(KC):
                        nc.tensor.matmul(
                            ps[:, 0:DG], vrow[kc][:, j * 128:(j + 1) * 128],
                            wv_t[:, kc * DG:(kc + 1) * DG],
                            start=(kc == 0), stop=False,
                        )
                    nc.tensor.matmul(ps[:, 0:DG], ones1_t[:], bv_t[:], start=False, stop=True)
                    nc.vector.tensor_copy(vp[:, j * DG:(j + 1) * DG], ps[:, 0:DG])

            # --- attention + output projection ----------------------------
            for c in range(NQC):
                nj = 4 * c + 4 if causal else NJ
                for p in range(2):
                    qoff = p * S + c * 512
                    ctx_ps = cpsum.tile([128, 512], f32, tag="ctx", name=f"ctx{c}{p}")
                    sum_ps = upsum.tile([128, 512], f32, tag="sum", name=f"sum{c}{p}")
                    for j in range(nj):
                        d = j - 4 * c if causal else -1
                        coff = 0 if d < 0 else 128 * d
                        sc = mmp.tile([128, 1024], f32, tag="sc", name=f"sc{c}{p}{j}")
                        for hh, (rlo, rhi) in enumerate(((0, 64), (64, 128))):
                            nc.tensor.matmul(
                                sc[:, hh * 512 + coff: hh * 512 + 512],
                                kpT[rlo:rhi, p * S + j * 128: p * S + (j + 1) * 128],
                                qpT[rlo:rhi, qoff + coff: qoff + 512],
                                start=True, stop=True, tile_position=(rlo, 0),
                            )
                        pr = probsp.tile([128, 1024], b16, tag="pr", name=f"pr{c}{p}{j}")
                        sc_v = sc.rearrange("p (h n) -> p h n", h=2)[:, :, coff:512]
                        pr_v = pr.rearrange("p (h n) -> p h n", h=2)[:, :, coff:512]
                        nc.scalar.activation(pr_v, sc_v, Exp, scale=1.0 / SCALE)
                        if d >= 0:
                            for hh in range(2):
                                band = pr[:, hh * 512 + coff: hh * 512 + coff + 128]
                                nc.vector.tensor_mul(band, band, tri_t[:])
                        first, last = (j == 0), (j == nj - 1)
                        for hh in range(2):
                            prh = pr[:, hh * 512 + coff: hh * 512 + 512]
                            nc.tensor.matmul(
                                ctx_ps[hh * 64:(hh + 1) * 64, coff:512],
                                vp[:, j * DG + p * 128 + hh * 64: j * DG + p * 128 + (hh + 1) * 64],
                                prh, start=first, stop=last,
                                tile_position=(0, hh * 64), skip_group_check=True,
                            )
                        for hh in range(2):
                            prh = pr[:, hh * 512 + coff: hh * 512 + 512]
                            nc.tensor.matmul(
                                sum_ps[hh * 64:(hh + 1) * 64, coff:512],
                                ones64_t[:], prh, start=first, stop=last,
                                tile_position=(0, hh * 64), skip_group_check=True,
                            )
                    rc_t = recp.tile([128, 512], f32, tag="rc", name=f"rc{c}{p}")
                    nc.vector.reciprocal_approx_fast(rc_t[:], sum_ps[:])
                    nc.vector.tensor_mul(ctxT[:, qoff: qoff + 512], ctx_ps[:], rc_t[:])
                for dc in range(KC):
                    opst = mmp.tile([128, 1024], f32, tag="sc", name=f"op{c}{dc}")
                    ops = opst[:, 0:512]
                    for p2 in range(2):
                        nc.tensor.matmul(
                            ops,
                            wo_t[:, p2 * D + dc * 128: p2 * D + (dc + 1) * 128],
                            ctxT[:, p2 * S + c * 512: p2 * S + (c + 1) * 512],
                            start=(p2 == 0), stop=(p2 == 1),
                        )
                    ot = ostp.tile([128, 512], f32, tag="ot", name=f"ot{c}{dc}")
                    nc.vector.tensor_copy(ot[:], ops)
                    nc.sync.dma_start(out[dc * 128:(dc + 1) * 128, c * 512:(c + 1) * 512], ot[:])

    nc.compile()
    return nc


def _get_nc(causal: bool):
    if causal not in _CACHE:
        _CACHE[causal] = _build(causal)
    return _CACHE[causal]


def make_in_maps(q, k, v, w_q, b_q, w_k, b_k, w_v, b_v, w_o):
    tri_keep = (np.arange(128)[:, None] <= np.arange(128)[None, :]).astype(BF16)
    qT = [np.ascontiguousarray(q[b].T).astype(BF16) for b in range(B)]
    kTn = [np.ascontiguousarray(k[b].T).astype(BF16) for b in range(B)]
    vTn = [np.ascontiguousarray(v[b].T).astype(BF16) for b in range(B)]
    in_maps = []
    for core in range(NCORES):
        b, g = core // G, core % G
        sl = slice(g * DG, (g + 1) * DG)
        in_maps.append({
            "qT": qT[b], "kT": kTn[b], "vT": vTn[b],
            "wqT": np.ascontiguousarray(w_q[sl, :].T).astype(BF16),
            "wkT": np.ascontiguousarray(w_k[sl, :].T).astype(BF16),
            "wvT": np.ascontiguousarray(w_v[sl, :].T).astype(BF16),
            "woT": np.ascontiguousarray(w_o[:, sl].T).astype(BF16),
            "bq": np.ascontiguousarray(b_q[sl, None]).astype(np.float32),
            "bk": np.ascontiguousarray(b_k[sl, None]).astype(np.float32),
            "bv": np.ascontiguousarray(b_v[None, sl]).astype(BF16),
            "tri": tri_keep,
        })
    return in_maps


def _reference_numpy(q, k, v, mask, w_q, b_q, w_k, b_k, w_v, b_v, w_o, b_o):
    qp = q @ w_q.T + b_q
    kp = k @ w_k.T + b_k
    vv = v @ w_v.T + b_v
    qp = qp.reshape(B, S, H, HD).transpose(0, 2, 1, 3)
    kp = kp.reshape(B, S, H, HD).transpose(0, 2, 1, 3)
    vv = vv.reshape(B, S, H, HD).transpose(0, 2, 1, 3)
    score = np.einsum("bhqd,bhkd->bhqk", qp, kp) / SCALE
    score = np.where(mask, -1e9, score)
    score -= score.max(axis=-1, keepdims=True)
    e = np.exp(score)
    attn = e / e.sum(axis=-1, keepdims=True)
    ctx = np.einsum("bhqk,bhkd->bhqd", attn, vv)
    ctx = ctx.transpose(0, 2, 1, 3).reshape(B, S, D)
    return (ctx @ w_o.T + b_o).astype(np.float32)


def kernel(q, k, v, mask, w_q, b_q, w_k, b_k, w_v, b_v, w_o, b_o):
    from concourse.bass_utils import run_bass_kernel_spmd

    q, k, v = (np.asarray(x, np.float32) for x in (q, k, v))
    mask = np.asarray(mask)
    causal_ref = np.triu(np.ones((S, S), bool), k=1)
    causal = all(np.array_equal(mask[b, 0], causal_ref) for b in range(B))
    if not causal and mask.any():
        # Unexpected mask pattern: fall back to exact numpy (never hit in
        # practice -- setup_inputs always builds the causal mask).
        return _reference_numpy(q, k, v, mask, w_q, b_q, w_k, b_k, w_v, b_v, w_o, b_o)

    nc = _get_nc(causal)
    in_maps = make_in_maps(q, k, v, w_q, b_q, w_k, b_k, w_v, b_v, w_o)
    res = run_bass_kernel_spmd(nc, in_maps, core_ids=list(range(NCORES)))

    out = np.zeros((B, S, D), np.float32)
    for core in range(NCORES):
        b = core // G
        out[b] += res.results[core]["out"].T
    out += np.asarray(b_o, np.float32)
    return out


# revision 13
# speedup vs baseline: 1.1505x; 1.1505x over previous
"""Multi-head attention (B=2,S=2048,D=1024,H=16) on 8 trn2 NeuronCores.

Sharding: core = b*4 + g  (b = batch 0..1, g = head-group 0..3, 4 heads each).
Each core computes QKV projections for its 256 output dims, causal attention
for its 4 heads (scores kept transposed: [s_k, s_q]), and a K-sliced partial
of the output projection (transposed: [D, S]).  Host sums the 4 partials per
batch and adds b_o.

All matmuls in bf16 (fp32 PSUM accumulate); softmax without max-subtraction
(scores/8 are small, exp cannot overflow); sumexp via an all-ones [128,64]
stationary matmul that also broadcasts the sum to all partitions of each
head's half, so normalization is a plain elementwise multiply.
"""
import sys

if "/opt/trn_rl_repo" not in sys.path:
    sys.path.insert(0, "/opt/trn_rl_repo")

import numpy as np
import ml_dtypes

B, S, D, H = 2, 2048, 1024, 16
HD = D // H            # 64
G = 4                  # head groups (one per core within a batch)
HPG = H // G           # 4 heads per group
DG = HPG * HD          # 256 dims per group
SCALE = 8.0
NCORES = 8
NQC = S // 512         # 4 query chunks
NJ = S // 128          # 16 key tiles
KC = D // 128          # 8 contraction chunks
BF16 = ml_dtypes.bfloat16

_CACHE = {}


def _build(causal: bool):
    import concourse.mybir as mybir
    import concourse.tile as tile
    from concourse import bacc

    f32 = mybir.dt.float32
    b16 = mybir.dt.bfloat16
    Exp = mybir.ActivationFunctionType.Exp

    nc = bacc.Bacc(None, target_bir_lowering=False)

    qT = nc.dram_tensor("qT", [D, S], b16, kind="ExternalInput")
    kT = nc.dram_tensor("kT", [D, S], b16, kind="ExternalInput")
    vT = nc.dram_tensor("vT", [D, S], b16, kind="ExternalInput")
    # weights host-prepacked to the exact SBUF tile layout (one DMA each)
    wqT = nc.dram_tensor("wqT", [128, KC * DG], b16, kind="ExternalInput")
    wkT = nc.dram_tensor("wkT", [128, KC * DG], b16, kind="ExternalInput")
    wvT = nc.dram_tensor("wvT", [128, KC * DG], b16, kind="ExternalInput")
    woT = nc.dram_tensor("woT", [128, 2 * D], b16, kind="ExternalInput")
    bq = nc.dram_tensor("bq", [128, 2], f32, kind="ExternalInput")
    bk = nc.dram_tensor("bk", [128, 2], f32, kind="ExternalInput")
    bv = nc.dram_tensor("bv", [1, DG], b16, kind="ExternalInput")
    tri = nc.dram_tensor("tri", [128, 128], b16, kind="ExternalInput")
    out = nc.dram_tensor("out", [D, S], b16, kind="ExternalOutput")

    with tile.TileContext(nc) as tc:
        with (
            tc.tile_pool(name="consts", bufs=1) as consts,
            tc.tile_pool(name="proj", bufs=1) as proj,
            tc.tile_pool(name="pin", bufs=1) as pin,
            tc.tile_pool(name="probs", bufs=6) as probsp,
            tc.tile_pool(name="rec", bufs=2) as recp,
            tc.tile_pool(name="ost", bufs=1) as ostp,
            tc.tile_pool(name="mm", bufs=3, space="PSUM") as mmp,
            tc.tile_pool(name="cpsum", bufs=1, space="PSUM") as cpsum,
            tc.tile_pool(name="upsum", bufs=1, space="PSUM") as upsum,
        ):
            # --- constant tiles -------------------------------------------
            wq_t = consts.tile([128, KC * DG], b16)
            wk_t = consts.tile([128, KC * DG], b16)
            wv_t = consts.tile([128, KC * DG], b16)
            wo_t = consts.tile([128, 2 * D], b16)
            bq_t = consts.tile([128, 2], f32)
            bk_t = consts.tile([128, 2], f32)
            bv_t = consts.tile([1, DG], b16)
            tri_t = consts.tile([128, 128], b16)
            ones64_t = consts.tile([128, HD], b16)
            nc.vector.memset(ones64_t[:], 1.0)
            ones1_t = consts.tile([1, 128], b16)
            nc.vector.memset(ones1_t[:], 1.0)
            warm_sb = consts.tile([128, 128], b16)
            nc.vector.memset(warm_sb[:], 0.0)

            # --- persistent projection outputs ----------------------------
            # qpT/kpT: pair p in cols [p*S,(p+1)*S); rows 0:64 head 2p, 64:128 head 2p+1
            qpT = proj.tile([128, 2 * S], b16)
            kpT = proj.tile([128, 2 * S], b16)
            # vp: key tile j in cols [j*DG,(j+1)*DG); within: local head hh at 64*hh
            vp = proj.tile([128, NJ * DG], b16)
            # ctxT: same pair layout as qpT, normalized attention output (c x s)
            ctxT = proj.tile([128, 2 * S], b16)

            # --- input row tiles + DMA schedule ---------------------------
            # quarter-split column DMAs so compute starts as soon as the
            # first 512 columns of each row land
            qrow = [pin.tile([128, S], b16, name=f"qrow{kc}") for kc in range(KC)]
            krow = [pin.tile([128, S], b16, name=f"krow{kc}") for kc in range(KC)]
            vrow = [pin.tile([128, S], b16, name=f"vrow{kc}") for kc in range(KC)]

            def dma_rows(rows, src, cs):
                for kc in range(KC):
                    nc.sync.dma_start(rows[kc][:, cs], src[kc * 128:(kc + 1) * 128, cs])

            first, rest = slice(0, 512), slice(512, S)
            nc.sync.dma_start(wq_t[:], wqT[:])
            dma_rows(qrow, qT, first)
            nc.sync.dma_start(wk_t[:], wkT[:])
            dma_rows(krow, kT, first)
            nc.sync.dma_start(wv_t[:], wvT[:])
            dma_rows(vrow, vT, first)
            nc.sync.dma_start(bq_t[:], bq[:])
            nc.sync.dma_start(bk_t[:], bk[:])
            nc.sync.dma_start(bv_t[:], bv[:])
            nc.sync.dma_start(tri_t[:], tri[:])
            nc.sync.dma_start(wo_t[:], woT[:])
            dma_rows(qrow, qT, rest)
            dma_rows(krow, kT, rest)
            dma_rows(vrow, vT, rest)

            # warmup burst: keeps the PE activity monitor at full clock
            # while the first input quarters stream in
            warm_ps = mmp.tile([128, 1024], f32, tag="sc", name="warm")
            for wi in range(36):
                nc.tensor.matmul(warm_ps[:, 0:128], warm_sb[:], warm_sb[:],
                                 start=(wi == 0), stop=(wi == 35))

            # --- projections, by column quarter ---------------------------
            for n in range(NQC):
                for name, rows, w_t, dst, bias_t in (
                    ("q", qrow, wq_t, qpT, bq_t),
                    ("k", krow, wk_t, kpT, bk_t),
                ):
                    for m in range(2):
                        ps = mmp.tile([128, 1024], f32, tag="sc", name=f"{name}ps{m}{n}")
                        for kc in range(KC):
                            nc.tensor.matmul(
                                ps[:, 0:512],
                                w_t[:, kc * DG + m * 128: kc * DG + (m + 1) * 128],
                                rows[kc][:, n * 512:(n + 1) * 512],
                                start=(kc == 0), stop=(kc == KC - 1),
                            )
                        nc.vector.tensor_scalar_add(
                            dst[:, m * S + n * 512: m * S + (n + 1) * 512],
                            ps[:, 0:512], bias_t[:, m:m + 1],
                        )
                for j in range(4 * n, 4 * n + 4):
                    ps = mmp.tile([128, 1024], f32, tag="sc", name=f"vps{j}")
                    for kc in range(KC):
                        nc.tensor.matmul(
                            ps[:, 0:DG], vrow[kc][:, j * 128:(j + 1) * 128],
                            wv_t[:, kc * DG:(kc + 1) * DG],
                            start=(kc == 0), stop=False,
                        )
                    nc.tensor.matmul(ps[:, 0:DG], ones1_t[:], bv_t[:], start=False, stop=True)
                    nc.vector.tensor_copy(vp[:, j * DG:(j + 1) * DG], ps[:, 0:DG])

            # --- attention + output projection ----------------------------
            ostage = {}
            for c in range(NQC):
                nj = 4 * c + 4 if causal else NJ
                for p in range(2):
                    qoff = p * S + c * 512
                    ctx_ps = cpsum.tile([128, 512], f32, tag="ctx", name=f"ctx{c}{p}")
                    sum_ps = upsum.tile([128, 512], f32, tag="sum", name=f"sum{c}{p}")
                    for j in range(nj):
                        d = j - 4 * c if causal else -1
                        coff = 0 if d < 0 else 128 * d
                        sc = mmp.tile([128, 1024], f32, tag="sc", name=f"sc{c}{p}{j}")
                        for hh, (rlo, rhi) in enumerate(((0, 64), (64, 128))):
                            nc.tensor.matmul(
                                sc[:, hh * 512 + coff: hh * 512 + 512],
                                kpT[rlo:rhi, p * S + j * 128: p * S + (j + 1) * 128],
                                qpT[rlo:rhi, qoff + coff: qoff + 512],
                                start=True, stop=True, tile_position=(rlo, 0),
                            )
                        pr = probsp.tile([128, 1024], b16, tag="pr", name=f"pr{c}{p}{j}")
                        sc_v = sc.rearrange("p (h n) -> p h n", h=2)[:, :, coff:512]
                        pr_v = pr.rearrange("p (h n) -> p h n", h=2)[:, :, coff:512]
                        nc.scalar.activation(pr_v, sc_v, Exp, scale=1.0 / SCALE)
                        if d >= 0:
                            for hh in range(2):
                                band = pr[:, hh * 512 + coff: hh * 512 + coff + 128]
                                nc.vector.tensor_mul(band, band, tri_t[:])
                        first, last = (j == 0), (j == nj - 1)
                        for hh in range(2):
                            prh = pr[:, hh * 512 + coff: hh * 512 + 512]
                            nc.tensor.matmul(
                                ctx_ps[hh * 64:(hh + 1) * 64, coff:512],
                                vp[:, j * DG + p * 128 + hh * 64: j * DG + p * 128 + (hh + 1) * 64],
                                prh, start=first, stop=last,
                                tile_position=(0, hh * 64), skip_group_check=True,
                            )
                        for hh in range(2):
                            prh = pr[:, hh * 512 + coff: hh * 512 + 512]
                            nc.tensor.matmul(
                                sum_ps[hh * 64:(hh + 1) * 64, coff:512],
                                ones64_t[:], prh, start=first, stop=last,
                                tile_position=(0, hh * 64), skip_group_check=True,
                            )
                    rc_t = recp.tile([128, 512], f32, tag="rc", name=f"rc{c}{p}")
                    nc.vector.reciprocal_approx_fast(rc_t[:], sum_ps[:])
                    nc.vector.tensor_mul(ctxT[:, qoff: qoff + 512], ctx_ps[:], rc_t[:])
                for dc in range(KC):
                    opst = mmp.tile([128, 1024], f32, tag="sc", name=f"op{c}{dc}")
                    ops = opst[:, 0:512]
                    for p2 in range(2):
                        nc.tensor.matmul(
                            ops,
                            wo_t[:, p2 * D + dc * 128: p2 * D + (dc + 1) * 128],
                            ctxT[:, p2 * S + c * 512: p2 * S + (c + 1) * 512],
                            start=(p2 == 0), stop=(p2 == 1),
                        )
                    # stage two 512-col chunks per output row-block so the
                    # out-DMA moves 2KB contiguous bursts (bf16 [128,1024])
                    if c % 2 == 0:
                        ostage[dc] = ostp.tile([128, 1024], b16, tag=f"ot{dc}", name=f"ot{c}{dc}")
                    nc.vector.tensor_copy(ostage[dc][:, (c % 2) * 512:(c % 2 + 1) * 512], ops)
                    if c % 2 == 1:
                        nc.sync.dma_start(
                            out[dc * 128:(dc + 1) * 128, (c - 1) * 512:(c + 1) * 512],
                            ostage[dc][:],
                        )

    nc.compile()
    return nc


def _get_nc(causal: bool):
    if causal not in _CACHE:
        _CACHE[causal] = _build(causal)
    return _CACHE[causal]


def _pack_w(w):
    # [D, DG] -> SBUF layout [128, KC*DG]: chunk kc of 128 rows side by side
    return np.ascontiguousarray(w.reshape(KC, 128, DG).transpose(1, 0, 2).reshape(128, KC * DG)).astype(BF16)


def make_in_maps(q, k, v, w_q, b_q, w_k, b_k, w_v, b_v, w_o):
    tri_keep = (np.arange(128)[:, None] <= np.arange(128)[None, :]).astype(BF16)
    qT = [np.ascontiguousarray(q[b].T).astype(BF16) for b in range(B)]
    kTn = [np.ascontiguousarray(k[b].T).astype(BF16) for b in range(B)]
    vTn = [np.ascontiguousarray(v[b].T).astype(BF16) for b in range(B)]
    in_maps = []
    for core in range(NCORES):
        b, g = core // G, core % G
        sl = slice(g * DG, (g + 1) * DG)
        woTg = np.ascontiguousarray(w_o[:, sl].T)  # [DG, D]
        in_maps.append({
            "qT": qT[b], "kT": kTn[b], "vT": vTn[b],
            "wqT": _pack_w(np.ascontiguousarray(w_q[sl, :].T)),
            "wkT": _pack_w(np.ascontiguousarray(w_k[sl, :].T)),
            "wvT": _pack_w(np.ascontiguousarray(w_v[sl, :].T)),
            "woT": np.ascontiguousarray(
                woTg.reshape(2, 128, D).transpose(1, 0, 2).reshape(128, 2 * D)).astype(BF16),
            "bq": np.ascontiguousarray(b_q[sl].reshape(2, 128).T).astype(np.float32),
            "bk": np.ascontiguousarray(b_k[sl].reshape(2, 128).T).astype(np.float32),
            "bv": np.ascontiguousarray(b_v[None, sl]).astype(BF16),
            "tri": tri_keep,
        })
    return in_maps


def _reference_numpy(q, k, v, mask, w_q, b_q, w_k, b_k, w_v, b_v, w_o, b_o):
    qp = q @ w_q.T + b_q
    kp = k @ w_k.T + b_k
    vv = v @ w_v.T + b_v
    qp = qp.reshape(B, S, H, HD).transpose(0, 2, 1, 3)
    kp = kp.reshape(B, S, H, HD).transpose(0, 2, 1, 3)
    vv = vv.reshape(B, S, H, HD).transpose(0, 2, 1, 3)
    score = np.einsum("bhqd,bhkd->bhqk", qp, kp) / SCALE
    score = np.where(mask, -1e9, score)
    score -= score.max(axis=-1, keepdims=True)
    e = np.exp(score)
    attn = e / e.sum(axis=-1, keepdims=True)
    ctx = np.einsum("bhqk,bhkd->bhqd", attn, vv)
    ctx = ctx.transpose(0, 2, 1, 3).reshape(B, S, D)
    return (ctx @ w_o.T + b_o).astype(np.float32)


def kernel(q, k, v, mask, w_q, b_q, w_k, b_k, w_v, b_v, w_o, b_o):
    from concourse.bass_utils import run_bass_kernel_spmd

    q, k, v = (np.asarray(x, np.float32) for x in (q, k, v))
    mask = np.asarray(mask)
    causal_ref = np.triu(np.ones((S, S), bool), k=1)
    causal = all(np.array_equal(mask[b, 0], causal_ref) for b in range(B))
    if not causal and mask.any():
        # Unexpected mask pattern: fall back to exact numpy (never hit in
        # practice -- setup_inputs always builds the causal mask).
        return _reference_numpy(q, k, v, mask, w_q, b_q, w_k, b_k, w_v, b_v, w_o, b_o)

    nc = _get_nc(causal)
    in_maps = make_in_maps(q, k, v, w_q, b_q, w_k, b_k, w_v, b_v, w_o)
    res = run_bass_kernel_spmd(nc, in_maps, core_ids=list(range(NCORES)))

    out = np.zeros((B, S, D), np.float32)
    for core in range(NCORES):
        b = core // G
        out[b] += res.results[core]["out"].T.astype(np.float32)
    out += np.asarray(b_o, np.float32)
    return out
